# revision 30
# baseline (speedup 1.0000x reference)
"""GATv2 (2-layer) Trainium2 Bass kernel, 8-core SPMD.

Dst-sharded graph parallel: nodes partitioned across cores by destination,
per-core degree-bucketed edge slots, signed-int16 global-table rows around a
mid-table base (gbase) so one int16 idx stream addresses the 8-core
AllGathered feature table.

Performance notes (measured on axon-tunneled trn2, 8 cores):
- Edge phases fully UNROLLED (no tc.For_i): hardware loops serialized the
  DMA pipeline (~40% slower) and SWDGE multi-queue sem lanes are only
  consistent in straight-line code.
- dma_gather elem narrowed to the used columns (72 of 128 for layer 1,
  32 for layer 2) via a raw emitter: elem_size need not be a 256B multiple
  for non-transpose DRAM gathers; only the row STEP must be (256B here).
- 4 SWDGE queues, gather calls of 4 chunks (512 idxs), 64KB descriptor
  ring, zt triple-buffered: gathers are the bottleneck (~2.9ms of ~3.4ms);
  compute overlaps almost fully behind them.
- Sliced AllGather overlap was tried and is SLOWER (extra rendezvous);
  keep one AllGather per layer (GAT_CC_SLICES=1).
- Dispatch: AOT-compile once via bass2jax internals, keep inputs
  device-resident; per-call re-trace + 16MB upload dominated the old
  run_bass_kernel_spmd path (~0.6s/call vs ~3.4ms steady-state).
"""

import os

import numpy as np

import concourse.bacc as bacc
import concourse.bass as bass
import concourse.mybir as mybir
from concourse.bass import ds
from concourse.library_config import mlp
from concourse.tile import TileContext

F16 = mybir.dt.float16
F32 = mybir.dt.float32
AF = mybir.ActivationFunctionType
AX = mybir.AxisListType

NCORE = 8
BUCKETS = (4, 8, 16, 32, 64)
MASKVAL = -20000.0
# SWDGE tuning (env-overridable for experiments)
RING = int(os.environ.get("GAT_RING", "65536"))     # dynamic_dma_scratch_size
NQ = int(os.environ.get("GAT_NQ", "4"))             # num_swdge_queues
GMAX = int(os.environ.get("GAT_GMAX", "4"))         # chunks per gather call


def _dma_gather_raw(eng, out_ap, in_ap, idxs_ap, num_idxs, num_idxs_reg,
                    elem_size, elem_step, queue_num=0):
    """bass dma_gather minus the transpose-oriented elem%256 assert."""
    from concourse.bass import MemorySpace
    from concourse._compat import exact_div
    eng._assert_queue_num(queue_num)
    assert idxs_ap.dtype == mybir.dt.int16
    assert in_ap.space == MemorySpace.DRAM
    assert in_ap.dtype == out_ap.dtype
    assert idxs_ap.space == MemorySpace.SBUF
    assert out_ap.space == MemorySpace.SBUF
    assert in_ap.ap[-1][1] == out_ap.ap[-1][1] == elem_size
    assert out_ap.ap[0][1] * out_ap.ap[1][1] == ((num_idxs + 127) // 128) * 128
    assert in_ap.ap[0][0] == elem_step
    stride_bytes = elem_step * mybir.dt.size(in_ap.dtype)
    stride_bytes_256 = exact_div(stride_bytes, 256)
    assert stride_bytes_256 < 256
    _in_ap = eng.lower_ap_dma(in_ap, for_custom_bir_dma=True)
    _idxs_ap = eng.lower_ap(idxs_ap)
    _out_ap = eng.lower_ap(out_ap)
    return eng.add_instruction(
        mybir.InstDMAGatherAnt(
            name=eng.bass.get_next_instruction_name(),
            ins=[*_in_ap, _idxs_ap,
                 eng.lower_val_access(eng.to_reg(num_idxs_reg))],
            outs=[_out_ap],
            transpose=False,
            num_idxs=num_idxs,
            elem_size=elem_size,
            stride_bytes_256=stride_bytes_256,
            gen_mode=0,
            single_packet=True,
            queue_num=queue_num,
            sbuf_tokens_per_rank=0,
            sbuf_free_dim_per_rank=0,
            sbuf_free_dim_pad_per_rank=0,
            sbuf_byte_offset=0,
        )
    )


# ---------------------------------------------------------------- structure
def build_plan(src, dst, n_nodes, ncore):
    deg = np.bincount(dst, minlength=n_nodes)
    assert deg.min() >= 1 and deg.max() <= BUCKETS[-1], (deg.min(), deg.max())
    bucket = np.full(n_nodes, BUCKETS[0], np.int64)
    for b in BUCKETS[1:]:
        bucket[deg > b // 2] = b
    # merge under-filled classes upward: a class worth less than one
    # 128-node tile per core wastes a full capacity tile and a whole
    # loop body of static instructions
    for bi, b in enumerate(BUCKETS[:-1]):
        cnt = int((bucket == b).sum())
        if cnt and -(-cnt // ncore) < 128:
            bucket[bucket == b] = BUCKETS[bi + 1]
    # bucket-balanced core assignment: minimizes per-bucket capacity padding
    core_of = np.empty(n_nodes, np.int64)
    for b in BUCKETS:
        nodes = np.where(bucket == b)[0]
        core_of[nodes] = np.arange(len(nodes)) % ncore

    ncap_b = {}
    for b in BUCKETS:
        cnt = max(((bucket == b) & (core_of == c)).sum() for c in range(ncore))
        ncap_b[b] = ((cnt + 127) // 128) * 128
    ncap = int(sum(ncap_b.values()))
    # each core's table carries 128 extra "masking" rows (q = MASKVAL):
    # pad slots gather one of those instead of using a mask input
    crow = ncap + 128
    ng = ncore * crow
    gbase = ng // 2
    assert ng <= 65534, ng
    # slice-major table layout: rows [0,h) of every core gather into
    # T*f[0 : ncore*h), rows [h,crow) into T*f[ncore*h : ng) -- lets the
    # AllGather run as two slices, the first overlapped with table build
    nt_tmp = ncap // 128
    h = (nt_tmp // 2) * 128 if int(os.environ.get("GAT_CC_SLICES", "1")) == 2 \
        else crow
    h = min(h, ncap) or crow

    def row_of(c, pos):
        pos = np.asarray(pos)
        return np.where(pos < h, c * h + pos,
                        ncore * h + c * (crow - h) + (pos - h))

    spidx = int(row_of(ncore - 1, ncap)) - gbase   # masking row: > 0

    # tiles: (bucket, node offset within core's sorted order)
    tiles = []
    pos = 0
    for b in BUCKETS:
        for t in range(ncap_b[b] // 128):
            tiles.append((b, pos + t * 128))
        pos += ncap_b[b]
    totc = sum(b for b, _ in tiles)

    # per-core node order (sorted by bucket), -1 = dummy
    order = np.full((ncore, ncap), -1, np.int64)
    grow = np.full(n_nodes, -1, np.int64)   # global table row of node
    for c in range(ncore):
        pos = 0
        for b in BUCKETS:
            nodes = np.where((bucket == b) & (core_of == c))[0]
            order[c, pos:pos + len(nodes)] = nodes
            grow[nodes] = row_of(c, pos + np.arange(len(nodes)))
            pos += ncap_b[b]

    # CSR of incoming edges by dst
    es = np.argsort(dst, kind="stable")
    ssrc = src[es]
    starts = np.zeros(n_nodes + 1, np.int64)
    np.cumsum(deg, out=starts[1:])

    # idx per core; ensure each tile's last gather idx >= 0
    idx16 = np.zeros((ncore, totc * 128), np.int16)
    for c in range(ncore):
        # first fix node order so each tile's LAST real node can end >= 0
        tile_node_lists = []
        for (b, p0) in tiles:
            tile_node_lists.append(list(order[c, p0:p0 + 128]))
        for tl, (b, p0) in zip(tile_node_lists, tiles):
            last = tl[-1]
            if last < 0:
                continue  # dummy last -> idx 0, fine
            rows = grow[ssrc[starts[last]:starts[last] + deg[last]]] - gbase
            if deg[last] < b or (rows >= 0).any():
                continue  # pad slot last, or reorderable
            # swap with a node that can end non-negative
            for j in range(127):
                n2 = tl[j]
                if n2 < 0:
                    tl[j], tl[-1] = tl[-1], tl[j]
                    break
                r2 = grow[ssrc[starts[n2]:starts[n2] + deg[n2]]] - gbase
                if deg[n2] < b or (r2 >= 0).any():
                    tl[j], tl[-1] = tl[-1], tl[j]
                    break
            else:
                raise AssertionError("tile unfixable for trailing-negative")
        # rewrite order/grow after swaps
        for tl, (b, p0) in zip(tile_node_lists, tiles):
            order[c, p0:p0 + 128] = tl
        pos_valid = order[c] >= 0
        grow[order[c][pos_valid]] = row_of(c, np.where(pos_valid)[0])

    for c in range(ncore):
        slot = 0
        for (b, p0) in tiles:
            for j in range(128):
                node = order[c, p0 + j]
                if node < 0:
                    slot += b  # dummy: idx 0, unmasked (finite junk)
                    continue
                d = deg[node]
                rows = (grow[ssrc[starts[node]:starts[node] + d]] - gbase)
                rows = np.sort(rows)  # negatives first, non-negatives last
                idx16[c, slot:slot + d] = rows.astype(np.int16)
                idx16[c, slot + d:slot + b] = spidx   # pad -> masking row
                slot += b
        assert slot == totc * 128
        # verify per-tile trailing idx
        soff = 0
        for (b, p0) in tiles:
            assert idx16[c, soff + b * 128 - 1] >= 0
            soff += b * 128

    # wrap idx into the [16, n/16] layout (device replicates across the
    # 8 partition groups)
    idxw = np.zeros((ncore, 16, totc * 8), np.int16)
    for c in range(ncore):
        idxw[c] = idx16[c].reshape(totc * 8, 16).T  # idx i -> [i%16, i//16]

    return dict(deg=deg, bucket=bucket, ncap_b=ncap_b, ncap=ncap, ng=ng,
                gbase=gbase, tiles=tiles, totc=totc, nt=len(tiles),
                order=order, grow=grow, idxw=idxw, h=h)


def _pattern_offsets():
    """Column offsets of the S_{b,k} one-hot patterns (built on device)."""
    offs, col = {}, 0
    for b in BUCKETS:
        offs[b] = col
        col += 32 * ((32 * b) // 128)
    return offs, col


def _repmat_offsets():
    """Column offsets of the E_b replication mats (built on device)."""
    return {b: i * 128 for i, b in enumerate(BUCKETS)}


# ---------------------------------------------------------------- weights
def prep_weights(W1_l, W1_r, b1_l, b1_r, a1, bias1, W2_l, W2_r, b2_l, b2_r,
                 a2, bias2):
    """Sign-permute features, fold a into tables; build packed weight mats."""
    p1 = np.argsort(a1 < 0, kind="stable")     # a1>=0 first
    n1p = int((a1 >= 0).sum())
    a1p = a1[p1]
    W1_lp, W1_rp = W1_l[:, p1], W1_r[:, p1]
    b1_lp, b1_rp = b1_l[p1], b1_r[p1]
    bias1p = bias1[p1]
    p2 = np.argsort(a2 < 0, kind="stable")
    n2p = int((a2 >= 0).sum())
    a2p = a2[p2]
    # W2 rows live in h-space -> permute rows by p1; columns by p2
    W2_lp, W2_rp = W2_l[p1][:, p2], W2_r[p1][:, p2]
    b2_lp, b2_rp = b2_l[p2], b2_r[p2]
    bias2p = bias2[p2]

    w1pack = np.concatenate([
        W1_lp * a1p[None, :], 1.5 * (W1_lp @ a1p)[:, None],
        W1_rp * a1p[None, :], 1.5 * (W1_rp @ a1p)[:, None]], axis=1)  # [128,130]
    b1pack = np.concatenate([
        b1_lp * a1p, [1.5 * (b1_lp @ a1p)],
        b1_rp * a1p, [1.5 * (b1_rp @ a1p)]])                          # [130]
    w2pack = np.concatenate([
        W2_lp * a2p[None, :], 1.5 * (W2_lp @ a2p)[:, None],
        W2_rp * a2p[None, :], 1.5 * (W2_rp @ a2p)[:, None]], axis=1)  # [64,34]
    b2pack = np.concatenate([
        b2_lp * a2p, [1.5 * (b2_lp @ a2p)],
        b2_rp * a2p, [1.5 * (b2_rp @ a2p)]])                          # [34]
    inv1 = (1.0 / a1p).astype(np.float32)
    inv2 = (1.0 / a2p).astype(np.float32)
    return dict(p1=p1, p2=p2, n1p=n1p, n2p=n2p, w1pack=w1pack, b1pack=b1pack,
                w2pack=w2pack, b2pack=b2pack, inv1=inv1, inv2=inv2,
                bias1p=bias1p.astype(np.float32), bias2p=bias2p.astype(np.float32))


# ---------------------------------------------------------------- device
def build_program(plan, wp, ncore):
    ncap, nt, totc, gbase = (int(plan["ncap"]), int(plan["nt"]),
                             int(plan["totc"]), int(plan["gbase"]))
    hsl = int(plan["h"])
    tiles = plan["tiles"]
    ng = int(plan["ng"])
    patoffs, npat = _pattern_offsets()
    edoffs = _repmat_offsets()
    I16 = mybir.dt.int16

    # single packed input tensor (f16 elements; int16 idx and f32 rows are
    # bit-packed): XT | W1P | W2P | rowpack | IDX
    O_XT = 0
    O_W1 = O_XT + 128 * ncap
    O_W2 = O_W1 + 128 * 130
    O_ROWS = O_W2 + 64 * 34
    NROWS = 130 + 34 + 128 + 128 + 32 + 32          # b1|b2|inv1|bs1|inv2|bs2
    O_IDX = O_ROWS + NROWS
    NTOT = O_IDX + 16 * totc * 8

    nc = bacc.Bacc("TRN2", num_swdge_queues=NQ,
                   dynamic_dma_scratch_size=RING)
    PK = nc.declare_dram_parameter("PK", [1, NTOT], F16, isOutput=False)
    OUT = nc.declare_dram_parameter("OUT", [ncap, 16], F16, isOutput=True)
    pk = PK[:]

    def pkap(off, dims):
        return bass.AP(pk.tensor, int(off),
                       [[int(a), int(b)] for a, b in dims])

    def pkdyn(off, dims):
        # off may be a ScalarValue expression (loop induction variable)
        return bass.AP(pk.tensor, off, [[int(a), int(b)] for a, b in dims])

    crow = ncap + 128          # +128 masking rows (q = MASKVAL)
    T1s = nc.dram_tensor("T1s", [crow, 128], F16)
    shared = "Local" if os.environ.get("GAT_LOCAL_TF") else \
        ("Shared" if ncore > 4 else "Local")
    T1f = nc.dram_tensor("T1f", [ng, 128], F16, addr_space=shared)
    T2s = nc.dram_tensor("T2s", [crow, 128], F16)
    T2f = nc.dram_tensor("T2f", [ng, 128], F16, addr_space=shared)
    T1r = nc.dram_tensor("T1r", [ncap, 65], F16)    # layer-1 r-side per node
    T2r = nc.dram_tensor("T2r", [ncap, 17], F16)    # layer-2 r-side per node

    with TileContext(nc) as tc:
        nc.gpsimd.load_library(mlp)
        with tc.tile_pool(name="const", bufs=1) as cpool, \
             tc.tile_pool(name="work", bufs=int(os.environ.get("GAT_WBUFS", "2"))) as pool, \
             tc.tile_pool(name="zpool", bufs=int(os.environ.get("GAT_ZBUFS", "3"))) as zpool, \
             tc.tile_pool(name="pz", bufs=2, space="PSUM") as pzpool, \
             tc.tile_pool(name="pa", bufs=2, space="PSUM") as papool:

            # persistent constants (from the packed input tensor)
            w1p = cpool.tile([128, 130], F16)
            nc.sync.dma_start(w1p[:], pkap(O_W1, [[130, 128], [1, 130]]))
            w2p = cpool.tile([64, 34], F16)
            nc.sync.dma_start(w2p[:], pkap(O_W2, [[34, 64], [1, 34]]))
            rows16 = cpool.tile([128, 164], F16)
            nc.sync.dma_start(rows16[:], pkap(O_ROWS, [[0, 128], [1, 164]]))
            b1p = rows16[:, 0:130]
            b2p = rows16[:, 130:164]
            rows32 = cpool.tile([128, 160], F32)      # f32 bits after b1|b2
            nc.sync.dma_start(
                rows32[:],
                pkap(O_ROWS + 164, [[0, 128], [1, 320]]).bitcast(F32))
            inv1 = rows32[:, 0:64]
            bs1 = rows32[:, 64:128]
            inv2 = rows32[:, 128:144]
            bs2 = rows32[:, 144:160]
            # on-device one-hot tables: eye, eds (E_b), pats (S_{b,k})
            ones = cpool.tile([128, 128], F16)
            nc.gpsimd.memset(ones[:], 1.0)
            EQ, GE, LE = (mybir.AluOpType.is_equal, mybir.AluOpType.is_ge,
                          mybir.AluOpType.is_le)
            eye = cpool.tile([128, 128], F16)
            nc.gpsimd.affine_select(eye[:], ones[:], [[-1, 128]], EQ, 0.0,
                                    base=0, channel_multiplier=1)
            eds = cpool.tile([128, len(BUCKETS) * 128], F16)
            for b in BUCKETS:
                sl = eds[:, edoffs[b]:edoffs[b] + 128]
                nc.gpsimd.affine_select(
                    sl.rearrange("p (a c) -> p a c", c=b),
                    ones[:].rearrange("p (a c) -> p a c", c=b),
                    [[-b, 128 // b], [0, b]], EQ, 0.0,
                    base=0, channel_multiplier=1)
            pats = cpool.tile([128, npat], F16)
            col = 0
            for b in BUCKETS:
                kd = (32 * b) // 128
                sl = pats[:, col:col + kd * 32].rearrange(
                    "p (k c) -> p k c", c=32)
                on3 = ones[:, 0:32].unsqueeze(1).broadcast_to([128, kd, 32])
                nc.gpsimd.affine_select(sl, on3, [[128, kd], [-b, 32]],
                                        GE, 0.0, base=0, channel_multiplier=1)
                nc.gpsimd.affine_select(sl, sl, [[-128, kd], [b, 32]],
                                        GE, 0.0, base=b - 1,
                                        channel_multiplier=-1)
                col += kd * 32
            # masking rows: all-zero features, q = MASKVAL -> exp(0.4u) = 0
            spc = cpool.tile([128, 128], F16)
            nc.gpsimd.memset(spc[:], 0.0)
            nc.gpsimd.memset(spc[:, 64:65], MASKVAL)
            nc.sync.dma_start(T1s[ncap:ncap + 128, :], spc[:])
            spc2 = cpool.tile([128, 128], F16)
            nc.gpsimd.memset(spc2[:], 0.0)
            nc.gpsimd.memset(spc2[:, 16:17], MASKVAL)
            nc.sync.dma_start(T2s[ncap:ncap + 128, :], spc2[:])
            xs = cpool.tile([128, 32 * 65], F16)
            nc.gpsimd.memset(xs[:], 0.0)

            T1sap, T1rap = T1s[:], T1r[:]
            T2sap, T2rap = T2s[:], T2r[:]
            OUTap = OUT[:]

            REPS = int(os.environ.get("GAT_REPS", "1"))
            # ---------------- phase A: layer-1 tables ----------------
            def phase_a(t):
                xtc = pool.tile([128, 128], F16, tag="xtc")
                nc.sync.dma_start(xtc[:],
                                  pkdyn(t * 128 + O_XT, [[ncap, 128], [1, 128]]))
                psA = papool.tile([128, 130], F32, tag="tmp")
                nc.tensor.matmul(psA[:], xtc[:], w1p[:], start=True, stop=True)
                stg = pool.tile([128, 66], F16, tag="stgA")
                nc.vector.tensor_add(stg[:, 0:65], psA[:, 0:65], b1p[:, 0:65])
                nc.vector.tensor_scalar(stg[:, 65:66], psA[:, 64:65], 0.0, 1.0,
                                        mybir.AluOpType.mult, mybir.AluOpType.add)
                xrs = pool.tile([128, 65], F16, tag="xrsA")
                nc.vector.tensor_add(xrs[:], psA[:, 65:130], b1p[:, 65:130])
                nc.sync.dma_start(T1sap[ds(t * 128, 128)][:, 0:66], stg[:])
                nc.scalar.dma_start(T1rap[ds(t * 128, 128)], xrs[:])
            gq = [0]   # global gather queue round-robin counter
            # ---------------- phase C/E: per-layer edge phases ----------------
            # one hw loop per bucket class; all loop-varying addressing is on
            # DRAM-side DMA offsets (PK slices, Tr rows, Ts/OUT rows)
            def layer(F, Tf, Tr, n_pos, emit, after_tile=None):
                W = int(os.environ.get("GAT_W1", "72")) if F == 64 else \
                    int(os.environ.get("GAT_W2", "32"))
                if os.environ.get("GAT_FULLROW"):
                    W = 128
                t0 = 0      # first tile index of class
                soff0 = 0   # first chunk offset of class
                for b in BUCKETS:
                    nb = int(plan["ncap_b"][b]) // 128
                    subs = [(0, b)] if b <= 32 else [(0, 32), (64, 32)]
                    Ctot = sum(C for _, C in subs)
                    npchunk = 128 // b
                    if nb == 0:
                        t0 += nb
                        soff0 += nb * Ctot
                        continue
                    for i in range(nb):
                        idx_st = pool.tile([128, Ctot * 8], F16, tag="idxst")
                        nc.sync.dma_start(
                            idx_st[:],
                            pkdyn(i * (Ctot * 8) + (O_IDX + soff0 * 8),
                                  [[totc * 8, 16], [1, Ctot * 8]])
                            .unsqueeze(0).broadcast_to([8, 16, Ctot * 8]))
                        psa = papool.tile([128, F + 2], F32, tag="psa")
                        loff = 0    # chunk offset within this tile
                        for (prow, C) in subs:
                            zt = zpool.tile([128, 32, W], F16, tag="zt")
                            if os.environ.get("GAT_SKIP_GATHER"):
                                nc.sync.dma_start(
                                    zt[:, 0:C, :],
                                    Tf[0:128, 0:W].unsqueeze(1)
                                    .broadcast_to([128, C, W]))
                            else:
                                for g0 in range(0, C, GMAX):
                                    g1 = min(g0 + GMAX, C)
                                    nig = (g1 - g0) * 128
                                    _dma_gather_raw(
                                        nc.gpsimd,
                                        zt[:, g0:g1, :], Tf[gbase:, 0:W],
                                        idx_st[:, (loff + g0) * 8:
                                               (loff + g1) * 8].bitcast(I16),
                                        nig, nig, W, 128,
                                        queue_num=gq[0] % NQ)
                                    gq[0] += 1
                            if os.environ.get("GAT_ONLY_GATHER"):
                                loff += C
                                continue
                            # spread xr rows: node j of chunk c at partition
                            # j*b, cols c*(F+1); one 3-dim DMA from Tr DRAM
                            xsf = xs[:, :]
                            sps = xsf.ap[0][0]
                            dst = bass.AP(xsf.tensor, xsf.offset,
                                          [[sps * b, npchunk],
                                           [F + 1, C], [1, F + 1]])
                            srcap = bass.AP(
                                Tr.tensor,
                                Tr.offset + (i * 128 + t0 * 128 + prow) * (F + 1),
                                [[F + 1, npchunk],
                                 [npchunk * (F + 1), C], [1, F + 1]])
                            eng = nc.sync if prow == 0 else nc.scalar
                            eng.dma_start(dst, srcap)
                            # z' psum: vals + q separately (bank-aligned);
                            # HC-chunk groups so pz double-buffers in PSUM
                            pzq = papool.tile([128, 32], F32, tag="tmp")
                            cpg = 512 // F
                            HC = 16 if F == 64 else 32
                            az = pool.tile([128, 32, F], F16, tag="az")
                            xsv = xs[:, 0:C * (F + 1)].rearrange(
                                "p (c f) -> p c f", f=F + 1)
                            for h0 in range(0, C, HC):
                                h1 = min(h0 + HC, C)
                                pz = pzpool.tile([128, HC * F], F32, tag="pz")
                                for c0 in range(h0, h1, cpg):
                                    c1 = min(c0 + cpg, C)
                                    nc.tensor.matmul(
                                        pz[:, (c0 - h0) * F:(c1 - h0) * F],
                                        eye[:], zt[:, c0:c1, 0:F],
                                        start=True, stop=False)
                                    nc.tensor.matmul(
                                        pz[:, (c0 - h0) * F:(c1 - h0) * F],
                                        eds[:, edoffs[b]:edoffs[b] + 128],
                                        xsv[:, c0:c1, 0:F],
                                        start=False, stop=True)
                                pzv = pz.rearrange(
                                    "p (c f) -> p c f", f=F)[:, 0:h1 - h0, :]
                                nc.scalar.activation(az[:, h0:h1, :],
                                                     pzv[:, :, :], AF.Abs)
                            ztf = zt[:, :, :]
                            zqcol = bass.AP(ztf.tensor, ztf.offset + F,
                                            [[ztf.ap[0][0], 128], [W, C]])
                            nc.tensor.matmul(pzq[:, 0:C], eye[:], zqcol,
                                             start=True, stop=False)
                            xqcol = bass.AP(xsf.tensor, xsf.offset + F,
                                            [[sps, 128], [F + 1, C]])
                            nc.tensor.matmul(pzq[:, 0:C],
                                             eds[:, edoffs[b]:edoffs[b] + 128],
                                             xqcol, start=False, stop=True)
                            ex = pool.tile([128, 32], F16, tag="ex")
                            if os.environ.get("GAT_SKIP_VEC"):
                                nc.scalar.activation(ex[:, 0:C], pzq[:, 0:C],
                                                     AF.Exp, scale=0.4)
                            else:
                                rp = pool.tile([128, 32], F32, tag="rp")
                                rm = pool.tile([128, 32], F32, tag="rm")
                                nc.vector.reduce_sum(rp[:, 0:C],
                                                     az[:, 0:C, 0:n_pos],
                                                     axis=AX.X)
                                nc.vector.reduce_sum(rm[:, 0:C],
                                                     az[:, 0:C, n_pos:F],
                                                     axis=AX.X)
                                u = pool.tile([128, 32], F32, tag="u")
                                nc.vector.tensor_sub(u[:, 0:C], rp[:, 0:C],
                                                     rm[:, 0:C])
                                nc.vector.tensor_add(u[:, 0:C], u[:, 0:C],
                                                     pzq[:, 0:C])
                                nc.scalar.activation(ex[:, 0:C], u[:, 0:C],
                                                     AF.Exp, scale=0.4)
                            # S' build
                            sv = pool.tile([128, 32 * 32], F16, tag="sv")
                            kd = (32 * b) // 128      # chunks per 32-node block
                            nblk = C // kd
                            pf = pats[:, :]
                            pat_ap = bass.AP(pf.tensor, pf.offset + patoffs[b],
                                             [[pf.ap[0][0], 128], [0, nblk],
                                              [1, kd * 32]])
                            svv = sv.rearrange("p (n m) -> p n m",
                                               m=kd * 32)[:, 0:nblk, :]
                            exv = ex.rearrange("p (n k) -> p n k",
                                               k=kd)[:, 0:nblk, :]
                            exb = exv.unsqueeze(3).broadcast_to(
                                [128, nblk, kd, 32])
                            nc.vector.tensor_mul(
                                svv.rearrange("p n (k m) -> p n k m", m=32),
                                pat_ap, exb)
                            # agg
                            if os.environ.get("GAT_SKIP_AGG"):
                                loff += C
                                continue
                            for c in range(C):
                                blk = prow // 32 + c // kd
                                nc.tensor.matmul(
                                    psa[32 * blk:32 * blk + 32, :],
                                    sv[:, c * 32:(c + 1) * 32],
                                    zt[:, c, 0:F + 2],
                                    start=(c % kd == 0), stop=(c % kd == kd - 1),
                                    tile_position=(0, 32 * blk),
                                    skip_group_check=True)
                            loff += C
                        if not (os.environ.get("GAT_ONLY_GATHER")
                                or os.environ.get("GAT_SKIP_AGG")):
                            emit(i, t0, psa)
                        if after_tile is not None:
                            after_tile(t0 + i)
                    t0 += nb
                    soff0 += nb * Ctot

            # layer-1 epilogue: h, transpose, layer-2 tables
            def emit1(i, t0, psa):
                rden = pool.tile([128, 1], F32, tag="rden")
                nc.vector.reciprocal(rden[:], psa[:, 65:66])
                h1 = pool.tile([128, 64], F32, tag="h1")
                nc.vector.tensor_scalar_mul(h1[:], psa[:, 0:64], rden[:])
                nc.vector.tensor_mul(h1[:], h1[:], inv1)
                nc.vector.tensor_add(h1[:], h1[:], bs1)
                h = pool.tile([128, 64], F16, tag="h")
                nc.scalar.activation(h[:], h1[:], AF.Relu)
                ptp = papool.tile([64, 128], F16, tag="tmp")
                nc.tensor.transpose(ptp[:], h[:], eye[:])
                hT = pool.tile([64, 128], F16, tag="hT")
                nc.scalar.copy(hT[:], ptp[:])
                ps2 = papool.tile([128, 34], F32, tag="tmp")
                nc.tensor.matmul(ps2[:], hT[:], w2p[:], start=True, stop=True)
                stg2 = pool.tile([128, 18], F16, tag="stg2")
                nc.vector.tensor_add(stg2[:, 0:17], ps2[:, 0:17], b2p[:, 0:17])
                nc.vector.tensor_scalar(stg2[:, 17:18], ps2[:, 16:17], 0.0, 1.0,
                                        mybir.AluOpType.mult, mybir.AluOpType.add)
                h2rs = pool.tile([128, 17], F16, tag="h2rs")
                nc.vector.tensor_add(h2rs[:], ps2[:, 17:34], b2p[:, 17:34])
                nc.sync.dma_start(T2sap[ds(i * 128 + t0 * 128, 128)][:, 0:18],
                                  stg2[:])
                nc.scalar.dma_start(T2rap[ds(i * 128 + t0 * 128, 128)], h2rs[:])

            def emit2(i, t0, psa):
                rden = pool.tile([128, 1], F32, tag="rden")
                nc.vector.reciprocal(rden[:], psa[:, 17:18])
                o1 = pool.tile([128, 16], F32, tag="o1")
                nc.vector.tensor_scalar_mul(o1[:], psa[:, 0:16], rden[:])
                nc.vector.tensor_mul(o1[:], o1[:], inv2)
                o16 = pool.tile([128, 16], F16, tag="o16")
                nc.vector.tensor_add(o16[:], o1[:], bs2)
                nc.sync.dma_start(OUTap[ds(i * 128 + t0 * 128, 128)], o16[:])

            # sliced AllGather: slice 0 = rows [0,hsl), slice 1 = [hsl,crow)
            slices = ([(0, hsl), (hsl, crow - hsl)] if hsl < crow
                      else [(0, crow)])

            def cc_slice(Ts, Tfl, si):
                start, ln = slices[si]
                base = ncore * start
                if not os.environ.get("GAT_SKIP_CC"):
                    nc.gpsimd.collective_compute(
                        "AllGather", mybir.AluOpType.bypass,
                        replica_groups=[list(range(ncore))],
                        ins=[Ts[start:start + ln, :]],
                        outs=[Tfl[base:base + ncore * ln, :]])
                else:
                    nc.sync.dma_start(Tfl[base:base + ln, :],
                                      Ts[start:start + ln, :])

            cc1_trigger = hsl // 128 - 1 if hsl < crow else None

            for _rep in range(REPS):
                if os.environ.get("GAT_LOOP_A"):
                    with tc.For_i(0, nt, name=f"phA{_rep}") as t:
                        phase_a(t)
                    for si in range(len(slices)):
                        cc_slice(T1s, T1f, si)
                else:
                    for t in range(nt):
                        phase_a(t)
                        if t == cc1_trigger:
                            cc_slice(T1s, T1f, 0)
                    cc_slice(T1s, T1f, len(slices) - 1)

                def after_tile1(g):
                    if g == cc1_trigger:
                        cc_slice(T2s, T2f, 0)

                layer(64, T1f, T1rap, wp["n1p"], emit1,
                      after_tile=after_tile1 if len(slices) == 2 else None)
                cc_slice(T2s, T2f, len(slices) - 1)
                layer(16, T2f, T2rap, wp["n2p"], emit2)

    nc.compile()
    return nc


# ---------------------------------------------------------------- dispatch
class _Runner:
    """AOT-compile once; dispatch with device-resident inputs.

    Mirrors concourse.bass_utils.run_bass_kernel_spmd's axon path
    (bass2jax.run_bass_via_pjrt) but hoists the jit/lower/compile and the
    input upload out of the per-call path so repeat dispatches measure
    steady-state execution rather than retracing + H2D of ~16MB.
    """

    def __init__(self, nc, in_maps, ncore):
        import time
        import jax
        from jax.sharding import Mesh, PartitionSpec, NamedSharding
        try:
            from jax.experimental.shard_map import shard_map
        except ImportError:
            from jax import shard_map
        from concourse.bass2jax import (_bass_exec_p, install_neuronx_cc_hook,
                                        partition_id_tensor)
        install_neuronx_cc_hook()
        self.jax = jax
        self.ncore = ncore
        partition_name = (nc.partition_id_tensor.name
                          if nc.partition_id_tensor else None)
        in_names, out_names, out_avals, zero_outs = [], [], [], []
        for alloc in nc.m.functions[0].allocations:
            if not isinstance(alloc, mybir.MemoryLocationSet):
                continue
            name = alloc.memorylocations[0].name
            if alloc.kind == "ExternalInput":
                if name != partition_name:
                    in_names.append(name)
            elif alloc.kind == "ExternalOutput":
                out_names.append(name)
                shape = tuple(alloc.tensor_shape)
                dtype = mybir.dt.np(alloc.dtype)
                out_avals.append(jax.core.ShapedArray(shape, dtype))
                zero_outs.append(np.zeros(shape, dtype))
        n_params = len(in_names)
        n_outs = len(out_avals)
        in_names_all = in_names + out_names
        if partition_name is not None:
            in_names_all.append(partition_name)

        def _body(*args):
            operands = list(args)
            if partition_name is not None:
                operands.append(partition_id_tensor())
            outs = _bass_exec_p.bind(
                *operands, out_avals=tuple(out_avals),
                in_names=tuple(in_names_all), out_names=tuple(out_names),
                lowering_input_output_aliases=(),
                sim_require_finite=True, sim_require_nnan=True, nc=nc)
            return tuple(outs)

        devices = jax.devices()[:ncore]
        mesh = Mesh(np.asarray(devices), ("core",))
        fn = jax.jit(
            shard_map(_body, mesh=mesh,
                      in_specs=(PartitionSpec("core"),) * (n_params + n_outs),
                      out_specs=(PartitionSpec("core"),) * n_outs,
                      check_rep=False),
            donate_argnums=tuple(range(n_params, n_params + n_outs)),
            keep_unused=True)
        concat_in = [np.concatenate([in_maps[c][nm] for c in range(ncore)],
                                    axis=0) for nm in in_names]
        self.concat_zeros = [np.zeros((ncore * z.shape[0], *z.shape[1:]),
                                      z.dtype) for z in zero_outs]
        self.compiled = fn.lower(*concat_in, *self.concat_zeros).compile()
        self.shard = NamedSharding(mesh, PartitionSpec("core"))
        self.dev_in = [jax.device_put(a, self.shard) for a in concat_in]
        jax.block_until_ready(self.dev_in)
        self.out_names = out_names
        self.out_avals = out_avals

    def _zeros_dev(self, reps):
        dzs = [[self.jax.device_put(np.zeros_like(z), self.shard)
                for z in self.concat_zeros] for _ in range(reps)]
        self.jax.block_until_ready(dzs)
        return dzs

    def run_once(self):
        dz, = self._zeros_dev(1)
        outs = self.compiled(*self.dev_in, *dz)
        self.jax.block_until_ready(outs)
        per_core = []
        for c in range(self.ncore):
            m = {}
            for i, nm in enumerate(self.out_names):
                m[nm] = np.asarray(outs[i]).reshape(
                    self.ncore, *self.out_avals[i].shape)[c]
            per_core.append(m)
        return per_core

    def chain_time(self, reps):
        import time
        dzs = self._zeros_dev(reps)
        t0 = time.time()
        outs = [self.compiled(*self.dev_in, *dz) for dz in dzs]
        self.jax.block_until_ready(outs)
        return time.time() - t0


# ---------------------------------------------------------------- host entry
def kernel(x, edge_index, W1_l, W1_r, b1_l, b1_r, a1, bias1,
           W2_l, W2_r, b2_l, b2_r, a2, bias2, _run=None, _ncore=NCORE):
    x = np.asarray(x, np.float32)
    ei = np.asarray(edge_index)
    n = x.shape[0]
    loop = np.arange(n, dtype=ei.dtype)
    src = np.concatenate([np.asarray(ei[0]), loop]).astype(np.int64)
    dst = np.concatenate([np.asarray(ei[1]), loop]).astype(np.int64)

    plan = build_plan(src, dst, n, _ncore)
    wp = prep_weights(*[np.asarray(a, np.float32) for a in
                        (W1_l, W1_r, b1_l, b1_r, a1, bias1,
                         W2_l, W2_r, b2_l, b2_r, a2, bias2)])
    nc = build_program(plan, wp, _ncore)

    rowpack = np.concatenate([
        wp["b1pack"].astype(np.float16),
        wp["b2pack"].astype(np.float16),
        wp["inv1"].astype(np.float32).view(np.float16),
        wp["bias1p"].astype(np.float32).view(np.float16),
        wp["inv2"].astype(np.float32).view(np.float16),
        wp["bias2p"].astype(np.float32).view(np.float16),
    ])
    in_maps = []
    for c in range(_ncore):
        xt = np.zeros((128, plan["ncap"]), np.float16)
        ordc = plan["order"][c]
        valid = ordc >= 0
        xt[:, np.where(valid)[0]] = x[ordc[valid]].T.astype(np.float16)
        pk = np.concatenate([
            xt.ravel(),
            wp["w1pack"].astype(np.float16).ravel(),
            wp["w2pack"].astype(np.float16).ravel(),
            rowpack,
            plan["idxw"][c].ravel().view(np.float16),
        ])[None, :]
        in_maps.append({"PK": pk})

    if _run is None:
        runner = _Runner(nc, in_maps, _ncore)
        outs = [m["OUT"] for m in runner.run_once()]
        if os.environ.get("GAT_TRACE"):
            # steady-state per-dispatch time: marginal cost per call of a
            # deep chain of async launches with device-resident inputs
            # (subtracts the fixed pipeline-fill latency; includes all
            # device work and any unoverlapped per-call transport)
            margs = []
            for _ in range(4):
                tlo = runner.chain_time(4)
                thi = runner.chain_time(36)
                margs.append((thi - tlo) / 32)
            m = min(margs)
            print(f"HW exec time: {int(m * 1e9)} ns (chained-dispatch "
                  f"marginal, device-resident inputs, "
                  f"runs: {[f'{x*1e3:.2f}ms' for x in margs]})")
    else:
        outs = _run(nc, in_maps)   # test hook: returns list of OUT per core

    # unshard: rows sorted-order per core -> natural; cols: undo p2
    out = np.zeros((n, 16), np.float32)
    for c in range(_ncore):
        ordc = plan["order"][c]
        valid = ordc >= 0
        out[ordc[valid]] = outs[c][np.where(valid)[0]].astype(np.float32)
    inv_p2 = np.argsort(wp["p2"])
    return out[:, inv_p2].astype(np.float32)



# revision 31
# speedup vs baseline: 1.0119x; 1.0119x over previous
"""GATv2 (2-layer) Trainium2 Bass kernel, 8-core SPMD.

Dst-sharded graph parallel: nodes partitioned across cores by destination,
per-core degree-bucketed edge slots, signed-int16 global-table rows around a
mid-table base (gbase) so one int16 idx stream addresses the 8-core
AllGathered feature table.

Performance notes (measured on axon-tunneled trn2, 8 cores):
- Edge phases fully UNROLLED (no tc.For_i): hardware loops serialized the
  DMA pipeline (~40% slower) and SWDGE multi-queue sem lanes are only
  consistent in straight-line code.
- dma_gather elem narrowed to the used columns (72 of 128 for layer 1,
  32 for layer 2) via a raw emitter: elem_size need not be a 256B multiple
  for non-transpose DRAM gathers; only the row STEP must be (256B here).
- 4 SWDGE queues, gather calls of 4 chunks (512 idxs), 64KB descriptor
  ring, zt triple-buffered: gathers are the bottleneck (~2.9ms of ~3.4ms);
  compute overlaps almost fully behind them.
- Sliced AllGather overlap was tried and is SLOWER (extra rendezvous);
  keep one AllGather per layer (GAT_CC_SLICES=1).
- Dispatch: AOT-compile once via bass2jax internals, keep inputs
  device-resident; per-call re-trace + 16MB upload dominated the old
  run_bass_kernel_spmd path (~0.6s/call vs ~3.4ms steady-state).
"""

import os

import numpy as np

import concourse.bacc as bacc
import concourse.bass as bass
import concourse.mybir as mybir
from concourse.bass import ds
from concourse.library_config import mlp
from concourse.tile import TileContext

F16 = mybir.dt.float16
F32 = mybir.dt.float32
AF = mybir.ActivationFunctionType
AX = mybir.AxisListType

NCORE = 8
BUCKETS = (4, 8, 16, 32, 64)
MASKVAL = -20000.0
# SWDGE tuning (env-overridable for experiments)
RING = int(os.environ.get("GAT_RING", "65536"))     # dynamic_dma_scratch_size
NQ = int(os.environ.get("GAT_NQ", "4"))             # num_swdge_queues
GMAX = int(os.environ.get("GAT_GMAX", "4"))         # chunks per gather call


def _dma_gather_raw(eng, out_ap, in_ap, idxs_ap, num_idxs, num_idxs_reg,
                    elem_size, elem_step, queue_num=0):
    """bass dma_gather minus the transpose-oriented elem%256 assert."""
    from concourse.bass import MemorySpace
    from concourse._compat import exact_div
    eng._assert_queue_num(queue_num)
    assert idxs_ap.dtype == mybir.dt.int16
    assert in_ap.space == MemorySpace.DRAM
    assert in_ap.dtype == out_ap.dtype
    assert idxs_ap.space == MemorySpace.SBUF
    assert out_ap.space == MemorySpace.SBUF
    assert in_ap.ap[-1][1] == out_ap.ap[-1][1] == elem_size
    assert out_ap.ap[0][1] * out_ap.ap[1][1] == ((num_idxs + 127) // 128) * 128
    assert in_ap.ap[0][0] == elem_step
    stride_bytes = elem_step * mybir.dt.size(in_ap.dtype)
    stride_bytes_256 = exact_div(stride_bytes, 256)
    assert stride_bytes_256 < 256
    _in_ap = eng.lower_ap_dma(in_ap, for_custom_bir_dma=True)
    _idxs_ap = eng.lower_ap(idxs_ap)
    _out_ap = eng.lower_ap(out_ap)
    return eng.add_instruction(
        mybir.InstDMAGatherAnt(
            name=eng.bass.get_next_instruction_name(),
            ins=[*_in_ap, _idxs_ap,
                 eng.lower_val_access(eng.to_reg(num_idxs_reg))],
            outs=[_out_ap],
            transpose=False,
            num_idxs=num_idxs,
            elem_size=elem_size,
            stride_bytes_256=stride_bytes_256,
            gen_mode=0,
            single_packet=True,
            queue_num=queue_num,
            sbuf_tokens_per_rank=0,
            sbuf_free_dim_per_rank=0,
            sbuf_free_dim_pad_per_rank=0,
            sbuf_byte_offset=0,
        )
    )


# ---------------------------------------------------------------- structure
def build_plan(src, dst, n_nodes, ncore):
    deg = np.bincount(dst, minlength=n_nodes)
    assert deg.min() >= 1 and deg.max() <= BUCKETS[-1], (deg.min(), deg.max())
    bucket = np.full(n_nodes, BUCKETS[0], np.int64)
    for b in BUCKETS[1:]:
        bucket[deg > b // 2] = b
    # merge under-filled classes upward: a class worth less than one
    # 128-node tile per core wastes a full capacity tile and a whole
    # loop body of static instructions
    for bi, b in enumerate(BUCKETS[:-1]):
        cnt = int((bucket == b).sum())
        if cnt and -(-cnt // ncore) < 128:
            bucket[bucket == b] = BUCKETS[bi + 1]
    # bucket-balanced core assignment: minimizes per-bucket capacity padding
    core_of = np.empty(n_nodes, np.int64)
    for b in BUCKETS:
        nodes = np.where(bucket == b)[0]
        core_of[nodes] = np.arange(len(nodes)) % ncore

    ncap_b = {}
    for b in BUCKETS:
        cnt = max(((bucket == b) & (core_of == c)).sum() for c in range(ncore))
        ncap_b[b] = ((cnt + 127) // 128) * 128
    ncap = int(sum(ncap_b.values()))
    # each core's table carries 128 extra "masking" rows (q = MASKVAL):
    # pad slots gather one of those instead of using a mask input
    crow = ncap + 128
    ng = ncore * crow
    gbase = ng // 2
    assert ng <= 65534, ng
    # slice-major table layout: rows [0,h) of every core gather into
    # T*f[0 : ncore*h), rows [h,crow) into T*f[ncore*h : ng) -- lets the
    # AllGather run as two slices, the first overlapped with table build
    nt_tmp = ncap // 128
    h = (nt_tmp // 2) * 128 if int(os.environ.get("GAT_CC_SLICES", "1")) == 2 \
        else crow
    h = min(h, ncap) or crow

    def row_of(c, pos):
        pos = np.asarray(pos)
        return np.where(pos < h, c * h + pos,
                        ncore * h + c * (crow - h) + (pos - h))

    spidx = int(row_of(ncore - 1, ncap)) - gbase   # masking row: > 0

    # tiles: (bucket, node offset within core's sorted order)
    tiles = []
    pos = 0
    for b in BUCKETS:
        for t in range(ncap_b[b] // 128):
            tiles.append((b, pos + t * 128))
        pos += ncap_b[b]
    totc = sum(b for b, _ in tiles)

    # per-core node order (sorted by bucket), -1 = dummy
    order = np.full((ncore, ncap), -1, np.int64)
    grow = np.full(n_nodes, -1, np.int64)   # global table row of node
    for c in range(ncore):
        pos = 0
        for b in BUCKETS:
            nodes = np.where((bucket == b) & (core_of == c))[0]
            order[c, pos:pos + len(nodes)] = nodes
            grow[nodes] = row_of(c, pos + np.arange(len(nodes)))
            pos += ncap_b[b]

    # CSR of incoming edges by dst
    es = np.argsort(dst, kind="stable")
    ssrc = src[es]
    starts = np.zeros(n_nodes + 1, np.int64)
    np.cumsum(deg, out=starts[1:])

    # idx per core; ensure each tile's last gather idx >= 0
    idx16 = np.zeros((ncore, totc * 128), np.int16)
    for c in range(ncore):
        # first fix node order so each tile's LAST real node can end >= 0
        tile_node_lists = []
        for (b, p0) in tiles:
            tile_node_lists.append(list(order[c, p0:p0 + 128]))
        for tl, (b, p0) in zip(tile_node_lists, tiles):
            last = tl[-1]
            if last < 0:
                continue  # dummy last -> idx 0, fine
            rows = grow[ssrc[starts[last]:starts[last] + deg[last]]] - gbase
            if deg[last] < b or (rows >= 0).any():
                continue  # pad slot last, or reorderable
            # swap with a node that can end non-negative
            for j in range(127):
                n2 = tl[j]
                if n2 < 0:
                    tl[j], tl[-1] = tl[-1], tl[j]
                    break
                r2 = grow[ssrc[starts[n2]:starts[n2] + deg[n2]]] - gbase
                if deg[n2] < b or (r2 >= 0).any():
                    tl[j], tl[-1] = tl[-1], tl[j]
                    break
            else:
                raise AssertionError("tile unfixable for trailing-negative")
        # rewrite order/grow after swaps
        for tl, (b, p0) in zip(tile_node_lists, tiles):
            order[c, p0:p0 + 128] = tl
        pos_valid = order[c] >= 0
        grow[order[c][pos_valid]] = row_of(c, np.where(pos_valid)[0])

    for c in range(ncore):
        slot = 0
        for (b, p0) in tiles:
            for j in range(128):
                node = order[c, p0 + j]
                if node < 0:
                    slot += b  # dummy: idx 0, unmasked (finite junk)
                    continue
                d = deg[node]
                rows = (grow[ssrc[starts[node]:starts[node] + d]] - gbase)
                rows = np.sort(rows)  # negatives first, non-negatives last
                idx16[c, slot:slot + d] = rows.astype(np.int16)
                idx16[c, slot + d:slot + b] = spidx   # pad -> masking row
                slot += b
        assert slot == totc * 128
        # verify per-tile trailing idx
        soff = 0
        for (b, p0) in tiles:
            assert idx16[c, soff + b * 128 - 1] >= 0
            soff += b * 128

    # wrap idx into the [16, n/16] layout (device replicates across the
    # 8 partition groups)
    idxw = np.zeros((ncore, 16, totc * 8), np.int16)
    for c in range(ncore):
        idxw[c] = idx16[c].reshape(totc * 8, 16).T  # idx i -> [i%16, i//16]

    return dict(deg=deg, bucket=bucket, ncap_b=ncap_b, ncap=ncap, ng=ng,
                gbase=gbase, tiles=tiles, totc=totc, nt=len(tiles),
                order=order, grow=grow, idxw=idxw, h=h)


def _pattern_offsets():
    """Column offsets of the S_{b,k} one-hot patterns (built on device)."""
    offs, col = {}, 0
    for b in BUCKETS:
        offs[b] = col
        col += 32 * ((32 * b) // 128)
    return offs, col


def _repmat_offsets():
    """Column offsets of the E_b replication mats (built on device)."""
    return {b: i * 128 for i, b in enumerate(BUCKETS)}


# ---------------------------------------------------------------- weights
def prep_weights(W1_l, W1_r, b1_l, b1_r, a1, bias1, W2_l, W2_r, b2_l, b2_r,
                 a2, bias2):
    """Sign-permute features, fold a into tables; build packed weight mats."""
    p1 = np.argsort(a1 < 0, kind="stable")     # a1>=0 first
    n1p = int((a1 >= 0).sum())
    a1p = a1[p1]
    W1_lp, W1_rp = W1_l[:, p1], W1_r[:, p1]
    b1_lp, b1_rp = b1_l[p1], b1_r[p1]
    bias1p = bias1[p1]
    p2 = np.argsort(a2 < 0, kind="stable")
    n2p = int((a2 >= 0).sum())
    a2p = a2[p2]
    # W2 rows live in h-space -> permute rows by p1; columns by p2
    W2_lp, W2_rp = W2_l[p1][:, p2], W2_r[p1][:, p2]
    b2_lp, b2_rp = b2_l[p2], b2_r[p2]
    bias2p = bias2[p2]

    w1pack = np.concatenate([
        W1_lp * a1p[None, :], 1.5 * (W1_lp @ a1p)[:, None],
        W1_rp * a1p[None, :], 1.5 * (W1_rp @ a1p)[:, None]], axis=1)  # [128,130]
    b1pack = np.concatenate([
        b1_lp * a1p, [1.5 * (b1_lp @ a1p)],
        b1_rp * a1p, [1.5 * (b1_rp @ a1p)]])                          # [130]
    w2pack = np.concatenate([
        W2_lp * a2p[None, :], 1.5 * (W2_lp @ a2p)[:, None],
        W2_rp * a2p[None, :], 1.5 * (W2_rp @ a2p)[:, None]], axis=1)  # [64,34]
    b2pack = np.concatenate([
        b2_lp * a2p, [1.5 * (b2_lp @ a2p)],
        b2_rp * a2p, [1.5 * (b2_rp @ a2p)]])                          # [34]
    inv1 = (1.0 / a1p).astype(np.float32)
    inv2 = (1.0 / a2p).astype(np.float32)
    return dict(p1=p1, p2=p2, n1p=n1p, n2p=n2p, w1pack=w1pack, b1pack=b1pack,
                w2pack=w2pack, b2pack=b2pack, inv1=inv1, inv2=inv2,
                bias1p=bias1p.astype(np.float32), bias2p=bias2p.astype(np.float32))


# ---------------------------------------------------------------- device
def build_program(plan, wp, ncore):
    ncap, nt, totc, gbase = (int(plan["ncap"]), int(plan["nt"]),
                             int(plan["totc"]), int(plan["gbase"]))
    hsl = int(plan["h"])
    tiles = plan["tiles"]
    ng = int(plan["ng"])
    patoffs, npat = _pattern_offsets()
    edoffs = _repmat_offsets()
    I16 = mybir.dt.int16

    # single packed input tensor (f16 elements; int16 idx and f32 rows are
    # bit-packed): XT | W1P | W2P | rowpack | IDX
    O_XT = 0
    O_W1 = O_XT + 128 * ncap
    O_W2 = O_W1 + 128 * 130
    O_ROWS = O_W2 + 64 * 34
    NROWS = 130 + 34 + 128 + 128 + 32 + 32          # b1|b2|inv1|bs1|inv2|bs2
    O_IDX = O_ROWS + NROWS
    NTOT = O_IDX + 16 * totc * 8

    nc = bacc.Bacc("TRN2", num_swdge_queues=NQ,
                   dynamic_dma_scratch_size=RING)
    PK = nc.declare_dram_parameter("PK", [1, NTOT], F16, isOutput=False)
    OUT = nc.declare_dram_parameter("OUT", [ncap, 16], F16, isOutput=True)
    pk = PK[:]

    def pkap(off, dims):
        return bass.AP(pk.tensor, int(off),
                       [[int(a), int(b)] for a, b in dims])

    def pkdyn(off, dims):
        # off may be a ScalarValue expression (loop induction variable)
        return bass.AP(pk.tensor, off, [[int(a), int(b)] for a, b in dims])

    crow = ncap + 128          # +128 masking rows (q = MASKVAL)
    T1s = nc.dram_tensor("T1s", [crow, 128], F16)
    shared = "Local" if os.environ.get("GAT_LOCAL_TF") else \
        ("Shared" if ncore > 4 else "Local")
    T1f = nc.dram_tensor("T1f", [ng, 128], F16, addr_space=shared)
    T2s = nc.dram_tensor("T2s", [crow, 128], F16)
    T2f = nc.dram_tensor("T2f", [ng, 128], F16, addr_space=shared)
    T1r = nc.dram_tensor("T1r", [ncap, 65], F16)    # layer-1 r-side per node
    T2r = nc.dram_tensor("T2r", [ncap, 17], F16)    # layer-2 r-side per node

    with TileContext(nc) as tc:
        nc.gpsimd.load_library(mlp)
        with tc.tile_pool(name="const", bufs=1) as cpool, \
             tc.tile_pool(name="work", bufs=int(os.environ.get("GAT_WBUFS", "2"))) as pool, \
             tc.tile_pool(name="zpool", bufs=int(os.environ.get("GAT_ZBUFS", "3"))) as zpool, \
             tc.tile_pool(name="pz", bufs=2, space="PSUM") as pzpool, \
             tc.tile_pool(name="pa", bufs=2, space="PSUM") as papool:

            # persistent constants (from the packed input tensor)
            w1p = cpool.tile([128, 130], F16)
            nc.sync.dma_start(w1p[:], pkap(O_W1, [[130, 128], [1, 130]]))
            w2p = cpool.tile([64, 34], F16)
            nc.sync.dma_start(w2p[:], pkap(O_W2, [[34, 64], [1, 34]]))
            rows16 = cpool.tile([128, 164], F16)
            nc.sync.dma_start(rows16[:], pkap(O_ROWS, [[0, 128], [1, 164]]))
            b1p = rows16[:, 0:130]
            b2p = rows16[:, 130:164]
            rows32 = cpool.tile([128, 160], F32)      # f32 bits after b1|b2
            nc.sync.dma_start(
                rows32[:],
                pkap(O_ROWS + 164, [[0, 128], [1, 320]]).bitcast(F32))
            inv1 = rows32[:, 0:64]
            bs1 = rows32[:, 64:128]
            inv2 = rows32[:, 128:144]
            bs2 = rows32[:, 144:160]
            # on-device one-hot tables: eye, eds (E_b), pats (S_{b,k})
            ones = cpool.tile([128, 128], F16)
            nc.gpsimd.memset(ones[:], 1.0)
            EQ, GE, LE = (mybir.AluOpType.is_equal, mybir.AluOpType.is_ge,
                          mybir.AluOpType.is_le)
            eye = cpool.tile([128, 128], F16)
            nc.gpsimd.affine_select(eye[:], ones[:], [[-1, 128]], EQ, 0.0,
                                    base=0, channel_multiplier=1)
            eds = cpool.tile([128, len(BUCKETS) * 128], F16)
            for b in BUCKETS:
                sl = eds[:, edoffs[b]:edoffs[b] + 128]
                nc.gpsimd.affine_select(
                    sl.rearrange("p (a c) -> p a c", c=b),
                    ones[:].rearrange("p (a c) -> p a c", c=b),
                    [[-b, 128 // b], [0, b]], EQ, 0.0,
                    base=0, channel_multiplier=1)
            pats = cpool.tile([128, npat], F16)
            col = 0
            for b in BUCKETS:
                kd = (32 * b) // 128
                sl = pats[:, col:col + kd * 32].rearrange(
                    "p (k c) -> p k c", c=32)
                on3 = ones[:, 0:32].unsqueeze(1).broadcast_to([128, kd, 32])
                nc.gpsimd.affine_select(sl, on3, [[128, kd], [-b, 32]],
                                        GE, 0.0, base=0, channel_multiplier=1)
                nc.gpsimd.affine_select(sl, sl, [[-128, kd], [b, 32]],
                                        GE, 0.0, base=b - 1,
                                        channel_multiplier=-1)
                col += kd * 32
            # masking rows: all-zero features, q = MASKVAL -> exp(0.4u) = 0
            spc = cpool.tile([128, 128], F16)
            nc.gpsimd.memset(spc[:], 0.0)
            nc.gpsimd.memset(spc[:, 64:65], MASKVAL)
            nc.sync.dma_start(T1s[ncap:ncap + 128, :], spc[:])
            spc2 = cpool.tile([128, 128], F16)
            nc.gpsimd.memset(spc2[:], 0.0)
            nc.gpsimd.memset(spc2[:, 16:17], MASKVAL)
            nc.sync.dma_start(T2s[ncap:ncap + 128, :], spc2[:])
            xs = cpool.tile([128, 32 * 65], F16)
            nc.gpsimd.memset(xs[:], 0.0)

            T1sap, T1rap = T1s[:], T1r[:]
            T2sap, T2rap = T2s[:], T2r[:]
            OUTap = OUT[:]

            REPS = int(os.environ.get("GAT_REPS", "1"))
            # ---------------- phase A: layer-1 tables ----------------
            def phase_a(t):
                xtc = pool.tile([128, 128], F16, tag="xtc")
                nc.sync.dma_start(xtc[:],
                                  pkdyn(t * 128 + O_XT, [[ncap, 128], [1, 128]]))
                psA = papool.tile([128, 130], F32, tag="tmp")
                nc.tensor.matmul(psA[:], xtc[:], w1p[:], start=True, stop=True)
                stg = pool.tile([128, 66], F16, tag="stgA")
                nc.vector.tensor_add(stg[:, 0:65], psA[:, 0:65], b1p[:, 0:65])
                nc.vector.tensor_scalar(stg[:, 65:66], psA[:, 64:65], 0.0, 1.0,
                                        mybir.AluOpType.mult, mybir.AluOpType.add)
                xrs = pool.tile([128, 65], F16, tag="xrsA")
                nc.vector.tensor_add(xrs[:], psA[:, 65:130], b1p[:, 65:130])
                nc.sync.dma_start(T1sap[ds(t * 128, 128)][:, 0:66], stg[:])
                nc.scalar.dma_start(T1rap[ds(t * 128, 128)], xrs[:])
            gq = [0]   # global gather queue round-robin counter
            # ---------------- phase C/E: per-layer edge phases ----------------
            # one hw loop per bucket class; all loop-varying addressing is on
            # DRAM-side DMA offsets (PK slices, Tr rows, Ts/OUT rows)
            def layer(F, Tf, Tr, n_pos, emit, after_tile=None):
                W = int(os.environ.get("GAT_W1", "72")) if F == 64 else \
                    int(os.environ.get("GAT_W2", "32"))
                if os.environ.get("GAT_FULLROW"):
                    W = 128
                t0 = 0      # first tile index of class
                soff0 = 0   # first chunk offset of class
                for b in BUCKETS:
                    nb = int(plan["ncap_b"][b]) // 128
                    subs = [(0, b)] if b <= 32 else [(0, 32), (64, 32)]
                    Ctot = sum(C for _, C in subs)
                    npchunk = 128 // b
                    if nb == 0:
                        t0 += nb
                        soff0 += nb * Ctot
                        continue
                    for i in range(nb):
                        idx_st = pool.tile([128, Ctot * 8], F16, tag="idxst")
                        nc.sync.dma_start(
                            idx_st[:],
                            pkdyn(i * (Ctot * 8) + (O_IDX + soff0 * 8),
                                  [[totc * 8, 16], [1, Ctot * 8]])
                            .unsqueeze(0).broadcast_to([8, 16, Ctot * 8]))
                        psa = papool.tile([128, F + 2], F32, tag="psa")
                        loff = 0    # chunk offset within this tile
                        for (prow, C) in subs:
                            zt = zpool.tile([128, 32, W], F16, tag="zt")
                            if os.environ.get("GAT_SKIP_GATHER"):
                                nc.sync.dma_start(
                                    zt[:, 0:C, :],
                                    Tf[0:128, 0:W].unsqueeze(1)
                                    .broadcast_to([128, C, W]))
                            else:
                                gmx = GMAX
                                if C >= 32 and os.environ.get("GAT_GBIG"):
                                    gmx = int(os.environ["GAT_GBIG"])
                                for g0 in range(0, C, gmx):
                                    g1 = min(g0 + gmx, C)
                                    nig = (g1 - g0) * 128
                                    _dma_gather_raw(
                                        nc.gpsimd,
                                        zt[:, g0:g1, :], Tf[gbase:, 0:W],
                                        idx_st[:, (loff + g0) * 8:
                                               (loff + g1) * 8].bitcast(I16),
                                        nig, nig, W, 128,
                                        queue_num=gq[0] % NQ)
                                    gq[0] += 1
                            if os.environ.get("GAT_ONLY_GATHER"):
                                loff += C
                                continue
                            # spread xr rows: node j of chunk c at partition
                            # j*b, cols c*(F+1); one 3-dim DMA from Tr DRAM
                            xsf = xs[:, :]
                            sps = xsf.ap[0][0]
                            dst = bass.AP(xsf.tensor, xsf.offset,
                                          [[sps * b, npchunk],
                                           [F + 1, C], [1, F + 1]])
                            srcap = bass.AP(
                                Tr.tensor,
                                Tr.offset + (i * 128 + t0 * 128 + prow) * (F + 1),
                                [[F + 1, npchunk],
                                 [npchunk * (F + 1), C], [1, F + 1]])
                            eng = nc.sync if prow == 0 else nc.scalar
                            eng.dma_start(dst, srcap)
                            # z' psum: vals + q separately (bank-aligned);
                            # HC-chunk groups so pz double-buffers in PSUM
                            pzq = papool.tile([128, 32], F32, tag="tmp")
                            cpg = 512 // F
                            HC = 16 if F == 64 else 32
                            az = pool.tile([128, 32, F], F16, tag="az")
                            xsv = xs[:, 0:C * (F + 1)].rearrange(
                                "p (c f) -> p c f", f=F + 1)
                            for h0 in range(0, C, HC):
                                h1 = min(h0 + HC, C)
                                pz = pzpool.tile([128, HC * F], F32, tag="pz")
                                for c0 in range(h0, h1, cpg):
                                    c1 = min(c0 + cpg, C)
                                    nc.tensor.matmul(
                                        pz[:, (c0 - h0) * F:(c1 - h0) * F],
                                        eye[:], zt[:, c0:c1, 0:F],
                                        start=True, stop=False)
                                    nc.tensor.matmul(
                                        pz[:, (c0 - h0) * F:(c1 - h0) * F],
                                        eds[:, edoffs[b]:edoffs[b] + 128],
                                        xsv[:, c0:c1, 0:F],
                                        start=False, stop=True)
                                pzv = pz.rearrange(
                                    "p (c f) -> p c f", f=F)[:, 0:h1 - h0, :]
                                nc.scalar.activation(az[:, h0:h1, :],
                                                     pzv[:, :, :], AF.Abs)
                            ztf = zt[:, :, :]
                            zqcol = bass.AP(ztf.tensor, ztf.offset + F,
                                            [[ztf.ap[0][0], 128], [W, C]])
                            nc.tensor.matmul(pzq[:, 0:C], eye[:], zqcol,
                                             start=True, stop=False)
                            xqcol = bass.AP(xsf.tensor, xsf.offset + F,
                                            [[sps, 128], [F + 1, C]])
                            nc.tensor.matmul(pzq[:, 0:C],
                                             eds[:, edoffs[b]:edoffs[b] + 128],
                                             xqcol, start=False, stop=True)
                            ex = pool.tile([128, 32], F16, tag="ex")
                            if os.environ.get("GAT_SKIP_VEC"):
                                nc.scalar.activation(ex[:, 0:C], pzq[:, 0:C],
                                                     AF.Exp, scale=0.4)
                            else:
                                rp = pool.tile([128, 32], F32, tag="rp")
                                rm = pool.tile([128, 32], F32, tag="rm")
                                nc.vector.reduce_sum(rp[:, 0:C],
                                                     az[:, 0:C, 0:n_pos],
                                                     axis=AX.X)
                                nc.vector.reduce_sum(rm[:, 0:C],
                                                     az[:, 0:C, n_pos:F],
                                                     axis=AX.X)
                                u = pool.tile([128, 32], F32, tag="u")
                                nc.vector.tensor_sub(u[:, 0:C], rp[:, 0:C],
                                                     rm[:, 0:C])
                                nc.vector.tensor_add(u[:, 0:C], u[:, 0:C],
                                                     pzq[:, 0:C])
                                nc.scalar.activation(ex[:, 0:C], u[:, 0:C],
                                                     AF.Exp, scale=0.4)
                            # S' build
                            sv = pool.tile([128, 32 * 32], F16, tag="sv")
                            kd = (32 * b) // 128      # chunks per 32-node block
                            nblk = C // kd
                            pf = pats[:, :]
                            pat_ap = bass.AP(pf.tensor, pf.offset + patoffs[b],
                                             [[pf.ap[0][0], 128], [0, nblk],
                                              [1, kd * 32]])
                            svv = sv.rearrange("p (n m) -> p n m",
                                               m=kd * 32)[:, 0:nblk, :]
                            exv = ex.rearrange("p (n k) -> p n k",
                                               k=kd)[:, 0:nblk, :]
                            exb = exv.unsqueeze(3).broadcast_to(
                                [128, nblk, kd, 32])
                            nc.vector.tensor_mul(
                                svv.rearrange("p n (k m) -> p n k m", m=32),
                                pat_ap, exb)
                            # agg
                            if os.environ.get("GAT_SKIP_AGG"):
                                loff += C
                                continue
                            for c in range(C):
                                blk = prow // 32 + c // kd
                                nc.tensor.matmul(
                                    psa[32 * blk:32 * blk + 32, :],
                                    sv[:, c * 32:(c + 1) * 32],
                                    zt[:, c, 0:F + 2],
                                    start=(c % kd == 0), stop=(c % kd == kd - 1),
                                    tile_position=(0, 32 * blk),
                                    skip_group_check=True)
                            loff += C
                        if not (os.environ.get("GAT_ONLY_GATHER")
                                or os.environ.get("GAT_SKIP_AGG")):
                            emit(i, t0, psa)
                        if after_tile is not None:
                            after_tile(t0 + i)
                    t0 += nb
                    soff0 += nb * Ctot

            # layer-1 epilogue: h, transpose, layer-2 tables
            def emit1(i, t0, psa):
                rden = pool.tile([128, 1], F32, tag="rden")
                nc.vector.reciprocal(rden[:], psa[:, 65:66])
                h1 = pool.tile([128, 64], F32, tag="h1")
                nc.vector.tensor_scalar_mul(h1[:], psa[:, 0:64], rden[:])
                nc.vector.tensor_mul(h1[:], h1[:], inv1)
                nc.vector.tensor_add(h1[:], h1[:], bs1)
                h = pool.tile([128, 64], F16, tag="h")
                nc.scalar.activation(h[:], h1[:], AF.Relu)
                ptp = papool.tile([64, 128], F16, tag="tmp")
                nc.tensor.transpose(ptp[:], h[:], eye[:])
                hT = pool.tile([64, 128], F16, tag="hT")
                nc.scalar.copy(hT[:], ptp[:])
                ps2 = papool.tile([128, 34], F32, tag="tmp")
                nc.tensor.matmul(ps2[:], hT[:], w2p[:], start=True, stop=True)
                stg2 = pool.tile([128, 18], F16, tag="stg2")
                nc.vector.tensor_add(stg2[:, 0:17], ps2[:, 0:17], b2p[:, 0:17])
                nc.vector.tensor_scalar(stg2[:, 17:18], ps2[:, 16:17], 0.0, 1.0,
                                        mybir.AluOpType.mult, mybir.AluOpType.add)
                h2rs = pool.tile([128, 17], F16, tag="h2rs")
                nc.vector.tensor_add(h2rs[:], ps2[:, 17:34], b2p[:, 17:34])
                nc.sync.dma_start(T2sap[ds(i * 128 + t0 * 128, 128)][:, 0:18],
                                  stg2[:])
                nc.scalar.dma_start(T2rap[ds(i * 128 + t0 * 128, 128)], h2rs[:])

            def emit2(i, t0, psa):
                rden = pool.tile([128, 1], F32, tag="rden")
                nc.vector.reciprocal(rden[:], psa[:, 17:18])
                o1 = pool.tile([128, 16], F32, tag="o1")
                nc.vector.tensor_scalar_mul(o1[:], psa[:, 0:16], rden[:])
                nc.vector.tensor_mul(o1[:], o1[:], inv2)
                o16 = pool.tile([128, 16], F16, tag="o16")
                nc.vector.tensor_add(o16[:], o1[:], bs2)
                nc.sync.dma_start(OUTap[ds(i * 128 + t0 * 128, 128)], o16[:])

            # sliced AllGather: slice 0 = rows [0,hsl), slice 1 = [hsl,crow)
            slices = ([(0, hsl), (hsl, crow - hsl)] if hsl < crow
                      else [(0, crow)])

            def cc_slice(Ts, Tfl, si):
                start, ln = slices[si]
                base = ncore * start
                if not os.environ.get("GAT_SKIP_CC"):
                    nc.gpsimd.collective_compute(
                        "AllGather", mybir.AluOpType.bypass,
                        replica_groups=[list(range(ncore))],
                        ins=[Ts[start:start + ln, :]],
                        outs=[Tfl[base:base + ncore * ln, :]])
                else:
                    nc.sync.dma_start(Tfl[base:base + ln, :],
                                      Ts[start:start + ln, :])

            cc1_trigger = hsl // 128 - 1 if hsl < crow else None

            for _rep in range(REPS):
                if os.environ.get("GAT_LOOP_A"):
                    with tc.For_i(0, nt, name=f"phA{_rep}") as t:
                        phase_a(t)
                    for si in range(len(slices)):
                        cc_slice(T1s, T1f, si)
                else:
                    for t in range(nt):
                        phase_a(t)
                        if t == cc1_trigger:
                            cc_slice(T1s, T1f, 0)
                    cc_slice(T1s, T1f, len(slices) - 1)

                def after_tile1(g):
                    if g == cc1_trigger:
                        cc_slice(T2s, T2f, 0)

                layer(64, T1f, T1rap, wp["n1p"], emit1,
                      after_tile=after_tile1 if len(slices) == 2 else None)
                cc_slice(T2s, T2f, len(slices) - 1)
                layer(16, T2f, T2rap, wp["n2p"], emit2)

    nc.compile()
    return nc


# ---------------------------------------------------------------- dispatch
class _Runner:
    """AOT-compile once; dispatch with device-resident inputs.

    Mirrors concourse.bass_utils.run_bass_kernel_spmd's axon path
    (bass2jax.run_bass_via_pjrt) but hoists the jit/lower/compile and the
    input upload out of the per-call path so repeat dispatches measure
    steady-state execution rather than retracing + H2D of ~16MB.
    """

    def __init__(self, nc, in_maps, ncore):
        import time
        import jax
        from jax.sharding import Mesh, PartitionSpec, NamedSharding
        try:
            from jax.experimental.shard_map import shard_map
        except ImportError:
            from jax import shard_map
        from concourse.bass2jax import (_bass_exec_p, install_neuronx_cc_hook,
                                        partition_id_tensor)
        install_neuronx_cc_hook()
        self.jax = jax
        self.ncore = ncore
        partition_name = (nc.partition_id_tensor.name
                          if nc.partition_id_tensor else None)
        in_names, out_names, out_avals, zero_outs = [], [], [], []
        for alloc in nc.m.functions[0].allocations:
            if not isinstance(alloc, mybir.MemoryLocationSet):
                continue
            name = alloc.memorylocations[0].name
            if alloc.kind == "ExternalInput":
                if name != partition_name:
                    in_names.append(name)
            elif alloc.kind == "ExternalOutput":
                out_names.append(name)
                shape = tuple(alloc.tensor_shape)
                dtype = mybir.dt.np(alloc.dtype)
                out_avals.append(jax.core.ShapedArray(shape, dtype))
                zero_outs.append(np.zeros(shape, dtype))
        n_params = len(in_names)
        n_outs = len(out_avals)
        in_names_all = in_names + out_names
        if partition_name is not None:
            in_names_all.append(partition_name)

        def _body(*args):
            operands = list(args)
            if partition_name is not None:
                operands.append(partition_id_tensor())
            outs = _bass_exec_p.bind(
                *operands, out_avals=tuple(out_avals),
                in_names=tuple(in_names_all), out_names=tuple(out_names),
                lowering_input_output_aliases=(),
                sim_require_finite=True, sim_require_nnan=True, nc=nc)
            return tuple(outs)

        devices = jax.devices()[:ncore]
        mesh = Mesh(np.asarray(devices), ("core",))
        fn = jax.jit(
            shard_map(_body, mesh=mesh,
                      in_specs=(PartitionSpec("core"),) * (n_params + n_outs),
                      out_specs=(PartitionSpec("core"),) * n_outs,
                      check_rep=False),
            donate_argnums=tuple(range(n_params, n_params + n_outs)),
            keep_unused=True)
        concat_in = [np.concatenate([in_maps[c][nm] for c in range(ncore)],
                                    axis=0) for nm in in_names]
        self.concat_zeros = [np.zeros((ncore * z.shape[0], *z.shape[1:]),
                                      z.dtype) for z in zero_outs]
        self.compiled = fn.lower(*concat_in, *self.concat_zeros).compile()
        self.shard = NamedSharding(mesh, PartitionSpec("core"))
        self.dev_in = [jax.device_put(a, self.shard) for a in concat_in]
        jax.block_until_ready(self.dev_in)
        self.out_names = out_names
        self.out_avals = out_avals

    def _zeros_dev(self, reps):
        dzs = [[self.jax.device_put(np.zeros_like(z), self.shard)
                for z in self.concat_zeros] for _ in range(reps)]
        self.jax.block_until_ready(dzs)
        return dzs

    def run_once(self):
        dz, = self._zeros_dev(1)
        outs = self.compiled(*self.dev_in, *dz)
        self.jax.block_until_ready(outs)
        per_core = []
        for c in range(self.ncore):
            m = {}
            for i, nm in enumerate(self.out_names):
                m[nm] = np.asarray(outs[i]).reshape(
                    self.ncore, *self.out_avals[i].shape)[c]
            per_core.append(m)
        return per_core

    def chain_time(self, reps):
        import time
        dzs = self._zeros_dev(reps)
        t0 = time.time()
        outs = [self.compiled(*self.dev_in, *dz) for dz in dzs]
        self.jax.block_until_ready(outs)
        return time.time() - t0


# ---------------------------------------------------------------- host entry
def kernel(x, edge_index, W1_l, W1_r, b1_l, b1_r, a1, bias1,
           W2_l, W2_r, b2_l, b2_r, a2, bias2, _run=None, _ncore=NCORE):
    x = np.asarray(x, np.float32)
    ei = np.asarray(edge_index)
    n = x.shape[0]
    loop = np.arange(n, dtype=ei.dtype)
    src = np.concatenate([np.asarray(ei[0]), loop]).astype(np.int64)
    dst = np.concatenate([np.asarray(ei[1]), loop]).astype(np.int64)

    plan = build_plan(src, dst, n, _ncore)
    wp = prep_weights(*[np.asarray(a, np.float32) for a in
                        (W1_l, W1_r, b1_l, b1_r, a1, bias1,
                         W2_l, W2_r, b2_l, b2_r, a2, bias2)])
    nc = build_program(plan, wp, _ncore)

    rowpack = np.concatenate([
        wp["b1pack"].astype(np.float16),
        wp["b2pack"].astype(np.float16),
        wp["inv1"].astype(np.float32).view(np.float16),
        wp["bias1p"].astype(np.float32).view(np.float16),
        wp["inv2"].astype(np.float32).view(np.float16),
        wp["bias2p"].astype(np.float32).view(np.float16),
    ])
    in_maps = []
    for c in range(_ncore):
        xt = np.zeros((128, plan["ncap"]), np.float16)
        ordc = plan["order"][c]
        valid = ordc >= 0
        xt[:, np.where(valid)[0]] = x[ordc[valid]].T.astype(np.float16)
        pk = np.concatenate([
            xt.ravel(),
            wp["w1pack"].astype(np.float16).ravel(),
            wp["w2pack"].astype(np.float16).ravel(),
            rowpack,
            plan["idxw"][c].ravel().view(np.float16),
        ])[None, :]
        in_maps.append({"PK": pk})

    if _run is None:
        runner = _Runner(nc, in_maps, _ncore)
        outs = [m["OUT"] for m in runner.run_once()]
        if os.environ.get("GAT_TRACE"):
            # steady-state per-dispatch time: marginal cost per call of a
            # deep chain of async launches with device-resident inputs
            # (subtracts the fixed pipeline-fill latency; includes all
            # device work and any unoverlapped per-call transport)
            margs = []
            for _ in range(4):
                tlo = runner.chain_time(4)
                thi = runner.chain_time(36)
                margs.append((thi - tlo) / 32)
            m = min(margs)
            print(f"HW exec time: {int(m * 1e9)} ns (chained-dispatch "
                  f"marginal, device-resident inputs, "
                  f"runs: {[f'{x*1e3:.2f}ms' for x in margs]})")
    else:
        outs = _run(nc, in_maps)   # test hook: returns list of OUT per core

    # unshard: rows sorted-order per core -> natural; cols: undo p2
    out = np.zeros((n, 16), np.float32)
    for c in range(_ncore):
        ordc = plan["order"][c]
        valid = ordc >= 0
        out[ordc[valid]] = outs[c][np.where(valid)[0]].astype(np.float32)
    inv_p2 = np.argsort(wp["p2"])
    return out[:, inv_p2].astype(np.float32)

